# revision 1
# baseline (speedup 1.0000x reference)
"""Mistral-style GQA attention block (B=1, S=2048, HID=4096, 32 q heads /
8 kv heads, head_dim=128, RoPE, causal) on 8 Trainium2 NeuronCores.

Sharding: tensor-parallel over heads. Core c owns q heads [4c, 4c+4) and
kv head c: Wq/Wk/Wv column-sharded, Wo row-sharded; the o_proj partial
products are summed on the host (the all-reduce of the TP scheme).

Device kernel layout notes (per core):
  All matmuls use natural operand layouts -- no on-device transposes of
  activations except V (16 small PE transposes):
    Q^T[d,m] = Wq_chunk.T @ X^T_chunk      (d on partitions)
    S^T[k,q] = (K^T chunk).T @ Q^T chunk   (softmax runs over partitions)
    colsums  = ones.T @ exp(S^T)           (PE reduction over partitions)
    O^T[d,q] = V_chunk.T @ exp(S^T)
    Y[m,n]   = (O^T chunk).T @ Wo chunk
  Causality is exploited by only computing lower-triangle k-tiles; the
  four distinct diagonal-staircase mask patterns are sliced from the
  provided attention_mask input on the host.
  float32r (full fp32 data, reduced-precision PE mode, 4x faster than
  plain fp32) is used for the large matmuls.
"""

import os
import numpy as np
from contextlib import ExitStack

import concourse.bass as bass
from concourse import bacc
import concourse.tile as tile
from concourse import mybir
from concourse.bass_utils import run_bass_kernel_spmd
from concourse.masks import make_identity

AF = mybir.ActivationFunctionType
F32 = mybir.dt.float32
F32R = mybir.dt.float32r

S = 2048          # sequence length
HID = 4096        # hidden size
D = 128           # head dim
NCORES = 8
HPC = 4           # q heads per core
DPC = HPC * D     # 512 q-proj columns per core
MC = 512          # seq chunk (free dim of most matmuls)
NKC = HID // 128  # 32 contraction chunks for projections
NJC = S // MC     # 4 q chunks
NMT = S // 128    # 16 seq tiles of 128
SCALE = float(1.0 / np.sqrt(D))
ROPE_THETA = 10000.0

LAST_RESULTS = None  # BassKernelResults of the most recent run (for test.py)


def _rope(nc, pool, out, src_ps, cos, sin, tag):
    """out = src*cos + rotate_half(src)*sin, all [128, MC]; src in PSUM.

    A single ACT copy drains the PSUM bank (frees it for the next
    accumulation group after one op); the 5-op rope chain then runs on
    DVE off the SBUF scratch, overlapped with the next chunk's matmuls.
    """
    lo = slice(0, 64)
    hi = slice(64, 128)
    src = pool.tile([128, MC], F32, tag=f"rsc{tag}", bufs=2, name=f"ropesrc{tag}")
    nc.scalar.activation(src, src_ps, AF.Copy)
    tmp = pool.tile([128, MC], F32, tag="ropetmp", bufs=2, name="ropetmp")
    # sin is HALF-SWAPPED on the host (sin_sw[d] = sin[(d+64)%128]) so both
    # SBUF inputs of each mul share a base partition (walrus constraint).
    nc.vector.tensor_mul(out, src, cos)
    nc.vector.tensor_mul(tmp[lo, :], src[hi, :], sin[hi, :])
    nc.vector.tensor_mul(tmp[hi, :], src[lo, :], sin[lo, :])
    nc.vector.tensor_sub(out[lo, :], out[lo, :], tmp[lo, :])
    nc.vector.tensor_add(out[hi, :], out[hi, :], tmp[hi, :])


def _emit(nc, xkv, wq, wo, cosT, sinT, maskT, ones_in, y, rscr, tc):
    with ExitStack() as ctx:
        const = ctx.enter_context(tc.tile_pool(name="const", bufs=1))

        ident = const.tile([128, 128], F32, name="ident")
        make_identity(nc, ident)
        ones_col = const.tile([128, 1], F32R, name="ones_col")
        nc.sync.dma_start(ones_col, ones_in)

        cos_sb = const.tile([D, S], F32, name="cos_sb")
        nc.sync.dma_start(cos_sb, cosT)
        sin_sb = const.tile([D, S], F32, name="sin_sb")
        nc.sync.dma_start(sin_sb, sinT)
        # mask_sb[:, MC*t + b][a] = mask(q=b, k=128*t+a): the 4 staircase
        # patterns used on the k-tiles straddling the causal diagonal.
        mask_sb = const.tile([128, 4 * MC], F32, name="mask_sb")
        for t in range(4):
            nc.sync.dma_start(mask_sb[:, MC * t:MC * (t + 1)],
                              maskT[128 * t:128 * (t + 1), :])

        qt = [const.tile([D, S], F32R, name=f"qt{h}") for h in range(HPC)]
        kt = const.tile([D, S], F32R, name="kt")
        vsb = const.tile([128, S], F32R, name="vsb")  # vsb[:, 128i:+128] = V rows 128i..

        # ---------------- Phase A: projections + RoPE + V transpose -------
        with tc.tile_pool(name="pa", bufs=1) as pa, \
             tc.tile_pool(name="pap", bufs=1, space="PSUM") as pap:
            FB = MC + 2 * D          # 768: one fused chunk
            NG = int(os.environ.get("KERNEL_XKV_GROUP", "1"))
            XB = int(os.environ.get("KERNEL_XKV_BUFS", "8"))
            GW = NG * FB             # one packed DMA group (NG kc chunks)
            wq_t = []
            for g in range(4):
                w = pa.tile([128, 8 * MC], F32R, tag=f"wq{g}", name=f"wq_t{g}")
                nc.sync.dma_start(w, wq[:, 8 * MC * g:8 * MC * (g + 1)])
                wq_t.append(w)
            for mc in range(NJC):
                ms = slice(MC * mc, MC * (mc + 1))
                ps_q = [pap.tile([128, MC], F32, tag=f"q{h}", name=f"ps_q{h}_{mc}")
                        for h in range(HPC)]
                ps_k = pap.tile([128, MC], F32, tag="k", name=f"ps_k_{mc}")
                ps_v = pap.tile([128, MC], F32, tag="v", name=f"ps_v_{mc}")
                ngrp = NKC // NG
                for kcg in range(ngrp):
                    big = pa.tile([128, GW], F32R, tag="xkv", bufs=XB,
                                  name=f"xkv_{mc}_{kcg}")
                    nc.sync.dma_start(big, xkv[:, GW * (ngrp * mc + kcg):
                                               GW * (ngrp * mc + kcg + 1)])
                    for c2 in range(NG):
                        kc = NG * kcg + c2
                        base = FB * c2
                        xt_ = big[:, base:base + MC]
                        wk_ = big[:, base + MC:base + MC + D]
                        wv_ = big[:, base + MC + D:base + FB]
                        wqc = wq_t[kc // 8][:, MC * (kc % 8):MC * (kc % 8 + 1)]
                        st = kc == 0
                        sp = kc == NKC - 1
                        for h in range(HPC):
                            nc.tensor.matmul(ps_q[h], wqc[:, D * h:D * (h + 1)],
                                             xt_, start=st, stop=sp)
                        nc.tensor.matmul(ps_k, wk_, xt_, start=st, stop=sp)
                        nc.tensor.matmul(ps_v, wv_, xt_, start=st, stop=sp)
                for h in range(HPC):
                    _rope(nc, pa, qt[h][:, ms], ps_q[h], cos_sb[:, ms], sin_sb[:, ms], h)
                _rope(nc, pa, kt[:, ms], ps_k, cos_sb[:, ms], sin_sb[:, ms], 'k')
                vt_ = pa.tile([128, MC], F32, tag="vt", bufs=2, name=f"vt_{mc}")
                nc.scalar.activation(vt_, ps_v, AF.Copy)
                for b in range(4):
                    ps_t = pap.tile([128, 128], F32, tag="tr", name=f"ps_tr_{mc}_{b}")
                    nc.tensor.transpose(ps_t, vt_[:, 128 * b:128 * (b + 1)], ident)
                    i = 4 * mc + b
                    nc.vector.tensor_copy(vsb[:, 128 * i:128 * (i + 1)], ps_t)

        # ---------------- Phase B: attention --------------------------------
        phases = os.environ.get("KERNEL_PHASES", "ABC")
        if "B" not in phases:
            return
        obc = ctx.enter_context(tc.tile_pool(name="obc", bufs=1))
        ot = [obc.tile([D, S], F32R, name=f"ot{h}") for h in range(HPC)]
        with tc.tile_pool(name="pb", bufs=1) as pb, \
             tc.tile_pool(name="pbp", bufs=1, space="PSUM") as pbp:
            for h in range(HPC):
                for jc in range(NJC):
                    qs = slice(MC * jc, MC * (jc + 1))
                    nk = 4 * jc + 4
                    ps_o = pbp.tile([128, MC], F32, tag="o", bufs=int(os.environ.get("KERNEL_O_BUFS","2")), name=f"ps_o_{h}_{jc}")
                    ps_sum = pbp.tile([1, MC], F32, tag="sum", bufs=2, name=f"ps_sum_{h}_{jc}")
                    for i in range(nk):
                        ks = slice(128 * i, 128 * (i + 1))
                        ps_s = pbp.tile([128, MC], F32, tag="s", bufs=int(os.environ.get("KERNEL_S_BUFS","4")), name=f"ps_s_{h}_{jc}_{i}")
                        nc.tensor.matmul(ps_s, kt[:, ks], qt[h][:, qs],
                                         start=True, stop=True)
                        tt = i - 4 * jc
                        if tt >= 0:
                            nc.vector.tensor_add(ps_s, ps_s,
                                                 mask_sb[:, MC * tt:MC * (tt + 1)])
                        ex = pb.tile([128, MC], F32R, tag="ex", bufs=int(os.environ.get("KERNEL_EX_BUFS","6")), name=f"ex_{h}_{jc}_{i}")
                        nc.scalar.activation(ex, ps_s, AF.Exp, scale=SCALE)
                        st = i == 0
                        sp = i == nk - 1
                        nc.tensor.matmul(ps_o, vsb[:, ks], ex, start=st, stop=sp)
                        nc.tensor.matmul(ps_sum, ones_col, ex, start=st, stop=sp)
                    recip = pb.tile([1, MC], F32, tag="recip", bufs=2, name=f"recip_{h}_{jc}")
                    nc.vector.reciprocal(recip, ps_sum)
                    # broadcast recip over partitions via a DRAM bounce (off PE)
                    scr = rscr[4 * h + jc]
                    nc.sync.dma_start(scr, recip)
                    bcast = pb.tile([128, MC], F32, tag="bcast", bufs=2, name=f"bcast_{h}_{jc}")
                    nc.sync.dma_start(bcast, scr.to_broadcast((128, MC)))
                    nc.vector.tensor_mul(ot[h][:, qs], ps_o, bcast)

        # ---------------- Phase C: o_proj (row-sharded partial) -------------
        if "C" not in phases:
            return
        with tc.tile_pool(name="pc", bufs=1) as pc, \
             tc.tile_pool(name="pcp", bufs=1, space="PSUM") as pcp:
            HH = HID // 2
            for half in range(2):
                wo_t = [[None] * 4 for _ in range(HPC)]
                for dc in range(HPC):
                    for nq in range(4):
                        w = pc.tile([128, 512], F32R, tag=f"wo{dc}_{nq}",
                                    name=f"wo_{half}_{dc}_{nq}")
                        nc.sync.dma_start(
                            w, wo[128 * dc:128 * (dc + 1),
                                  HH * half + 512 * nq:HH * half + 512 * (nq + 1)])
                        wo_t[dc][nq] = w
                for mt in range(NMT):
                    yrow = pc.tile([128, HH], F32, tag="yrow", bufs=int(os.environ.get("KERNEL_YROW_BUFS","4")),
                                   name=f"yrow_{half}_{mt}")
                    for nq in range(4):
                        ps_y = pcp.tile([128, 512], F32, tag="y", bufs=int(os.environ.get("KERNEL_Y_BUFS","8")),
                                        name=f"ps_y_{half}_{mt}_{nq}")
                        for dc in range(HPC):
                            nc.tensor.matmul(ps_y, ot[dc][:, 128 * mt:128 * (mt + 1)],
                                             wo_t[dc][nq], start=(dc == 0),
                                             stop=(dc == HPC - 1))
                        nc.scalar.activation(yrow[:, 512 * nq:512 * (nq + 1)],
                                             ps_y, AF.Copy)
                    nc.sync.dma_start(y[128 * mt:128 * (mt + 1),
                                        HH * half:HH * (half + 1)], yrow)


_BUILT = None


def _build():
    global _BUILT
    if _BUILT is not None:
        return _BUILT
    nc = bacc.Bacc("TRN2", target_bir_lowering=False, debug=False,
                   num_devices=NCORES)
    xkv = nc.dram_tensor("xkv", [128, NJC * NKC * (MC + 2 * D)], F32R,
                         kind="ExternalInput").ap()
    wq = nc.dram_tensor("wq", [128, NKC * MC], F32R, kind="ExternalInput").ap()
    wo = nc.dram_tensor("wo", [DPC, HID], F32R, kind="ExternalInput").ap()
    cosT = nc.dram_tensor("cosT", [D, S], F32, kind="ExternalInput").ap()
    sinT = nc.dram_tensor("sinT", [D, S], F32, kind="ExternalInput").ap()
    maskT = nc.dram_tensor("maskT", [MC, MC], F32, kind="ExternalInput").ap()
    ones_in = nc.dram_tensor("ones_in", [128, 1], F32R, kind="ExternalInput").ap()
    y = nc.dram_tensor("y", [S, HID], F32, kind="ExternalOutput").ap()
    rscr = [nc.dram_tensor(f"rscr{i}", [1, MC], F32).ap() for i in range(16)]
    with tile.TileContext(nc) as tc:
        _emit(nc, xkv, wq, wo, cosT, sinT, maskT, ones_in, y, rscr, tc)
    nc.compile()
    _BUILT = nc
    return nc


def prep_in_maps(hidden_states, Wq, Wk, Wv, Wo, attention_mask, position_ids):
    hidden_states = np.asarray(hidden_states, dtype=np.float32)
    Wq = np.asarray(Wq, dtype=np.float32)
    Wk = np.asarray(Wk, dtype=np.float32)
    Wv = np.asarray(Wv, dtype=np.float32)
    Wo = np.asarray(Wo, dtype=np.float32)
    attention_mask = np.asarray(attention_mask, dtype=np.float32)
    position_ids = np.asarray(position_ids)

    xT = np.ascontiguousarray(hidden_states[0].T)  # [HID, S]

    # RoPE tables (host-precomputed from position_ids, as in the reference)
    pos = position_ids[0].astype(np.float32)  # [S]
    inv_freq = (1.0 / (ROPE_THETA ** (np.arange(0, D, 2, dtype=np.float32) / D))
                ).astype(np.float32)
    freqs = pos[:, None] * inv_freq[None, :]           # [S, D/2]
    emb = np.concatenate([freqs, freqs], axis=-1)      # [S, D]
    cosT = np.ascontiguousarray(np.cos(emb).T.astype(np.float32))  # [D, S]
    sinT = np.sin(emb).T.astype(np.float32)
    sinT = np.ascontiguousarray(np.concatenate([sinT[64:], sinT[:64]], axis=0))

    # diagonal staircase mask patterns, sliced from the provided mask
    maskT = np.ascontiguousarray(attention_mask[0, 0, :MC, :MC].T)  # [k, q]

    xTr = xT.reshape(NKC, 128, S)
    in_maps = []
    for c in range(NCORES):
        wk_c = Wk[:, D * c:D * (c + 1)].reshape(NKC, 128, D)
        wv_c = Wv[:, D * c:D * (c + 1)].reshape(NKC, 128, D)
        # blocks[mc, kc, p, j]: fused chunk = [xT cols | Wk | Wv]
        blocks = np.empty((NJC, NKC, 128, MC + 2 * D), dtype=np.float32)
        for mc in range(NJC):
            blocks[mc, :, :, :MC] = xTr[:, :, MC * mc:MC * (mc + 1)]
            blocks[mc, :, :, MC:MC + D] = wk_c
            blocks[mc, :, :, MC + D:] = wv_c
        # -> [p, mc, kc, j] flattened to the packed DMA layout
        xkv = blocks.transpose(2, 0, 1, 3).reshape(128, -1)
        wq_c = (Wq[:, DPC * c:DPC * (c + 1)].reshape(NKC, 128, DPC)
                .transpose(1, 0, 2).reshape(128, -1))
        in_maps.append({
            "xkv": np.ascontiguousarray(xkv),
            "wq": np.ascontiguousarray(wq_c),
            "wo": np.ascontiguousarray(Wo[DPC * c:DPC * (c + 1), :]),
            "cosT": cosT,
            "sinT": sinT,
            "maskT": maskT,
            "ones_in": np.ones((128, 1), dtype=np.float32),
        })

    return in_maps


def kernel(hidden_states, Wq, Wk, Wv, Wo, attention_mask, position_ids):
    global LAST_RESULTS
    in_maps = prep_in_maps(hidden_states, Wq, Wk, Wv, Wo, attention_mask,
                           position_ids)
    nc = _build()
    res = run_bass_kernel_spmd(nc, in_maps, list(range(NCORES)),
                               trace=bool(int(os.environ.get("KERNEL_TRACE", "0"))))
    LAST_RESULTS = res

    acc = np.zeros((S, HID), dtype=np.float64)
    for c in range(NCORES):
        acc += res.results[c]["y"].astype(np.float64)
    return acc.astype(np.float32)[None]  # [1, S, HID]



# revision 13
# speedup vs baseline: 1.2965x; 1.2965x over previous
"""Mistral-style GQA attention block (B=1, S=2048, HID=4096, 32 q heads /
8 kv heads, head_dim=128, RoPE, causal) on 8 Trainium2 NeuronCores.

Sharding: tensor-parallel over heads. Core c owns q heads [4c, 4c+4) and
kv head c: Wq/Wk/Wv column-sharded, Wo row-sharded; the o_proj partial
products are summed on the host (the all-reduce of the TP scheme).

Device kernel layout notes (per core):
  All matmuls use natural operand layouts -- no on-device transposes of
  activations except V (16 small PE transposes):
    Q^T[d,m] = Wq_chunk.T @ X^T_chunk      (d on partitions)
    S^T[k,q] = (K^T chunk).T @ Q^T chunk   (softmax runs over partitions)
    den      = ones.T @ exp(S^T)           (PE reduction over partitions;
                                            ones is [128,128] so the
                                            denominator lands broadcast
                                            across all 128 partitions --
                                            no separate broadcast step)
    O^T[d,q] = V_chunk.T @ exp(S^T)
    Y[m,n]   = (O^T chunk).T @ Wo chunk
  Causality: only lower-triangle k-tiles are computed; diagonal tiles are
  fixed up by multiplying exp(S^T) with a 0/1 staircase mask on DVE (keeps
  the mask off PSUM and off the S->exp critical path).

  Projections and o_proj run in residual-split fp8 (DoubleRow perf mode,
  256-deep contraction at 0.5 PE cycles/row): X@W ~= Xhi@Whi + Xhi@Wlo +
  Xlo16@(Whi/16), with weights pre-scaled by 64 on the host and the 1/64
  folded into the PSUM-draining activation copies. Attention itself
  (RoPE'd Q/K, V, exp tiles) runs in bf16.
"""

import os
import numpy as np
from contextlib import ExitStack

import ml_dtypes

import concourse.bass as bass
from concourse import bacc
import concourse.tile as tile
from concourse import mybir
from concourse.bass_utils import run_bass_kernel_spmd
from concourse.masks import make_identity

AF = mybir.ActivationFunctionType
DR = mybir.MatmulPerfMode.DoubleRow
F32 = mybir.dt.float32
BF16 = mybir.dt.bfloat16
FP8 = mybir.dt.float8e4
NP_BF16 = ml_dtypes.bfloat16
NP_FP8 = ml_dtypes.float8_e4m3

S = 2048          # sequence length
HID = 4096        # hidden size
D = 128           # head dim
NCORES = 8
HPC = 4           # q heads per core
DPC = HPC * D     # 512 q-proj columns per core
MC = 512          # seq chunk (free dim of most matmuls)
NKC = HID // 128  # 32 contraction chunks for projections
NPAIR = NKC // 2  # 16 DoubleRow contraction pairs for projections
NJC = S // MC     # 4 q chunks
NMT = S // 128    # 16 seq tiles of 128
SCALE = float(1.0 / np.sqrt(D))
WS = 64.0         # fp8 weight pre-scale (folded back in PSUM drains)
ROPE_THETA = 10000.0

# fp8 phase-A pair-block layout (bytes per partition row)
XHI, XLO = 0, 1024
KW, VW = 2048, 2816   # 3 x [2,128] versions each: hi, lo, hi/16
PBLK = 3584
# bf16 phase-A chunk layout
FB = MC + 2 * D       # 768: one fused xkv chunk (x | wk | wv)

LAST_RESULTS = None  # BassKernelResults of the most recent run (for test.py)


def _env(name, dflt):
    return int(os.environ.get(name, str(dflt)))


FP8A = bool(_env("KERNEL_FP8A", 1))
FP8C = bool(_env("KERNEL_FP8C", 1))


def _rope(nc, pool, out, src_ps, cos, sin, tag, scale):
    """out = src*cos + rotate_half(src)*sin, all [128, MC]; src in PSUM.

    A single ACT copy drains the PSUM bank (frees it for the next
    accumulation group after one op); the 5-op rope chain then runs on
    DVE in bf16 (2x DVE mode), overlapped with the next chunk's matmuls.
    """
    lo = slice(0, 64)
    hi = slice(64, 128)
    src = pool.tile([128, MC], BF16, tag=f"rsc{tag}", bufs=2, name=f"ropesrc{tag}")
    nc.scalar.activation(src, src_ps, AF.Copy, scale=scale)
    tmp = pool.tile([128, MC], BF16, tag="ropetmp", bufs=2, name="ropetmp")
    # sin is HALF-SWAPPED on the host (sin_sw[d] = sin[(d+64)%128]) so both
    # SBUF inputs of each mul share a base partition (walrus constraint).
    nc.vector.tensor_mul(out, src, cos)
    nc.vector.tensor_mul(tmp[lo, :], src[hi, :], sin[hi, :])
    nc.vector.tensor_mul(tmp[hi, :], src[lo, :], sin[lo, :])
    nc.vector.tensor_sub(out[lo, :], out[lo, :], tmp[lo, :])
    nc.vector.tensor_add(out[hi, :], out[hi, :], tmp[hi, :])


def _pair3(ap1024):
    """[128, 1024] slice -> [128, 2, 512] DoubleRow operand view."""
    return ap1024.rearrange("p (j n) -> p j n", j=2)


def _emit(nc, xkv, wq, wo, cosT, sinT, maskT, y, tc):
    NG = _env("KERNEL_XKV_GROUP", 4 if not FP8A else 2)
    XB = _env("KERNEL_XKV_BUFS", 3)
    SB = _env("KERNEL_S_BUFS", 3)
    OB = _env("KERNEL_O_BUFS", 2)
    DB = _env("KERNEL_DEN_BUFS", 1)
    YB = _env("KERNEL_Y_BUFS", 2)
    EXB = _env("KERNEL_EX_BUFS", 6)
    TRB = _env("KERNEL_TR_BUFS", 2)
    YRB = _env("KERNEL_YROW_BUFS", 2)
    nchunk = NPAIR if FP8A else NKC           # contraction steps per mc
    CW = PBLK if FP8A else FB                 # cols per contraction step
    GW = NG * CW                              # cols per DMA group
    ngrp = nchunk // NG
    drain_scale = (1.0 / WS) if FP8A else 1.0

    with ExitStack() as ctx:
        const = ctx.enter_context(tc.tile_pool(name="const", bufs=1))

        ident = const.tile([128, 128], BF16, name="ident")
        make_identity(nc, ident)
        ones_sq = const.tile([128, 128], BF16, name="ones_sq")
        nc.gpsimd.memset(ones_sq, 1.0)

        cos_sb = const.tile([D, S], BF16, name="cos_sb")
        sin_sb = const.tile([D, S], BF16, name="sin_sb")
        # mask_sb[:, MC*t + b][a] = 0/1 keep-mask(q=b, k=128*t+a): the 4
        # staircase patterns for the k-tiles straddling the causal diagonal,
        # applied multiplicatively to exp(S^T).
        mask_sb = const.tile([128, 4 * MC], BF16, name="mask_sb")
        if FP8C:
            wo_sb = const.tile([128, 2 * 8 * 3 * 1024], FP8, name="wo_sb")
        else:
            wo_sb = const.tile([128, HPC * 8 * 512], BF16, name="wo_sb")

        qt = [const.tile([D, S], BF16, name=f"qt{h}") for h in range(HPC)]
        kt = const.tile([D, S], BF16, name="kt")
        vsb = const.tile([128, S], BF16, name="vsb")  # vsb[:, 128i:+128] = V rows 128i..
        if FP8C:
            othi = const.tile([128, HPC, S], FP8, name="othi")
            otlo = const.tile([128, HPC, S], FP8, name="otlo")
            ot = None
        else:
            ot = [const.tile([D, S], BF16, name=f"ot{h}") for h in range(HPC)]

        # ---------------- Phase A: projections + RoPE + V transpose -------
        with tc.tile_pool(name="pa", bufs=1) as pa, \
             tc.tile_pool(name="pap", bufs=1, space="PSUM") as pap:
            # wq lives in the phase-A pool so its SBUF is recycled for the
            # B/C scratch tiles.
            if FP8A:
                wq_sb = pa.tile([128, NPAIR * 3072], FP8, name="wq_sb")
            else:
                wq_sb = pa.tile([128, NKC * MC], BF16, name="wq_sb")

            # Deferred const DMAs: interleaved with the first xkv group DMAs
            # so the projection stream never waits behind a bulk upfront
            # transfer.
            wq_cols = wq_sb.shape[1]
            NWQ = 8

            def wq_dma(j):
                def emit():
                    w = wq_cols // NWQ
                    nc.sync.dma_start(wq_sb[:, w * j:w * (j + 1)],
                                      wq[:, w * j:w * (j + 1)])
                return emit

            def mask_dma(t):
                def emit():
                    nc.sync.dma_start(mask_sb[:, MC * t:MC * (t + 1)],
                                      maskT[128 * t:128 * (t + 1), :])
                return emit

            def wo_dma(j):
                def emit():
                    half = wo_sb.shape[1] // 2
                    nc.sync.dma_start(wo_sb[:, half * j:half * (j + 1)],
                                      wo[:, half * j:half * (j + 1)])
                return emit

            pending = ([wq_dma(j) for j in range(1, NWQ)]
                       + [lambda: nc.sync.dma_start(cos_sb, cosT),
                          lambda: nc.sync.dma_start(sin_sb, sinT)]
                       + [mask_dma(t) for t in range(4)]
                       + [wo_dma(0), wo_dma(1)])
            wq_dma(0)()
            for mc in range(NJC):
                ms = slice(MC * mc, MC * (mc + 1))
                ps_q = [pap.tile([128, MC], F32, tag=f"q{h}", name=f"ps_q{h}_{mc}")
                        for h in range(HPC)]
                ps_k = pap.tile([128, MC], F32, tag="k", name=f"ps_k_{mc}")
                ps_v = pap.tile([128, MC], F32, tag="v", name=f"ps_v_{mc}")
                for g in range(ngrp):
                    big = pa.tile([128, GW], FP8 if FP8A else BF16, tag="xkv",
                                  bufs=XB, name=f"xkv_{mc}_{g}")
                    nc.sync.dma_start(big, xkv[:, GW * (ngrp * mc + g):
                                               GW * (ngrp * mc + g + 1)])
                    if pending:
                        pending.pop(0)()
                    for c2 in range(NG):
                        ck = NG * g + c2
                        base = CW * c2
                        st = ck == 0
                        sp = ck == nchunk - 1
                        if FP8A:
                            xh = _pair3(big[:, base + XHI:base + XHI + 1024])
                            xl = _pair3(big[:, base + XLO:base + XLO + 1024])
                            wk3 = [big[:, base + KW + 256 * v:
                                       base + KW + 256 * (v + 1)]
                                   .rearrange("p (j n) -> p j n", j=2)
                                   for v in range(3)]
                            wv3 = [big[:, base + VW + 256 * v:
                                       base + VW + 256 * (v + 1)]
                                   .rearrange("p (j n) -> p j n", j=2)
                                   for v in range(3)]
                            for h in range(HPC):
                                hsl = slice(128 * h, 128 * (h + 1))
                                wq3 = [_pair3(wq_sb[:, 3072 * ck + 1024 * v:
                                                    3072 * ck + 1024 * (v + 1)]
                                              )[:, :, hsl] for v in range(3)]
                                nc.tensor.matmul(ps_q[h], wq3[0], xh,
                                                 start=st, stop=False,
                                                 perf_mode=DR)
                                nc.tensor.matmul(ps_q[h], wq3[1], xh,
                                                 start=False, stop=False,
                                                 perf_mode=DR)
                                nc.tensor.matmul(ps_q[h], wq3[2], xl,
                                                 start=False, stop=sp,
                                                 perf_mode=DR)
                            for ps, w3 in ((ps_k, wk3), (ps_v, wv3)):
                                nc.tensor.matmul(ps, w3[0], xh, start=st,
                                                 stop=False, perf_mode=DR)
                                nc.tensor.matmul(ps, w3[1], xh, start=False,
                                                 stop=False, perf_mode=DR)
                                nc.tensor.matmul(ps, w3[2], xl, start=False,
                                                 stop=sp, perf_mode=DR)
                        else:
                            xt_ = big[:, base:base + MC]
                            wk_ = big[:, base + MC:base + MC + D]
                            wv_ = big[:, base + MC + D:base + FB]
                            wqc = wq_sb[:, MC * ck:MC * (ck + 1)]
                            for h in range(HPC):
                                nc.tensor.matmul(ps_q[h],
                                                 wqc[:, D * h:D * (h + 1)],
                                                 xt_, start=st, stop=sp)
                            nc.tensor.matmul(ps_k, wk_, xt_, start=st, stop=sp)
                            nc.tensor.matmul(ps_v, wv_, xt_, start=st, stop=sp)
                # V path first: the PE transposes only wait on the ACT copy,
                # never on the DVE rope backlog.
                vt_ = pa.tile([128, MC], BF16, tag="vt", bufs=2, name=f"vt_{mc}")
                nc.scalar.activation(vt_, ps_v, AF.Copy, scale=drain_scale)
                for b in range(4):
                    ps_t = pap.tile([128, 128], BF16, tag="tr", bufs=TRB,
                                    name=f"ps_tr_{mc}_{b}")
                    nc.tensor.transpose(ps_t, vt_[:, 128 * b:128 * (b + 1)], ident)
                    i = 4 * mc + b
                    nc.vector.tensor_copy(vsb[:, 128 * i:128 * (i + 1)], ps_t)
                for h in range(HPC):
                    _rope(nc, pa, qt[h][:, ms], ps_q[h], cos_sb[:, ms],
                          sin_sb[:, ms], h, drain_scale)
                _rope(nc, pa, kt[:, ms], ps_k, cos_sb[:, ms], sin_sb[:, ms],
                      'k', drain_scale)

        # ---------------- Phases B+C interleaved --------------------------
        phases = os.environ.get("KERNEL_PHASES", "ABC")
        if "B" not in phases:
            return
        with tc.tile_pool(name="pb", bufs=1) as pb, \
             tc.tile_pool(name="pbp", bufs=1, space="PSUM") as pbp:

            def B(jc):
                qs = slice(MC * jc, MC * (jc + 1))
                nk = 4 * jc + 4
                for h in range(HPC):
                    ps_o = pbp.tile([128, MC], F32, tag="o", bufs=OB,
                                    name=f"ps_o_{h}_{jc}")
                    ps_den = pbp.tile([128, MC], F32, tag="den", bufs=DB,
                                      name=f"ps_den_{h}_{jc}")
                    for i in range(nk):
                        ks = slice(128 * i, 128 * (i + 1))
                        ps_s = pbp.tile([128, MC], F32, tag="s", bufs=SB,
                                        name=f"ps_s_{h}_{jc}_{i}")
                        nc.tensor.matmul(ps_s, kt[:, ks], qt[h][:, qs],
                                         start=True, stop=True)
                        ex = pb.tile([128, MC], BF16, tag="ex", bufs=EXB,
                                     name=f"ex_{h}_{jc}_{i}")
                        nc.scalar.activation(ex, ps_s, AF.Exp, scale=SCALE)
                        tt = i - 4 * jc
                        if tt >= 0:
                            nc.vector.tensor_mul(ex, ex,
                                                 mask_sb[:, MC * tt:MC * (tt + 1)])
                        st = i == 0
                        sp = i == nk - 1
                        nc.tensor.matmul(ps_o, vsb[:, ks], ex, start=st, stop=sp)
                        nc.tensor.matmul(ps_den, ones_sq, ex, start=st, stop=sp)
                    recip = pb.tile([128, MC], F32, tag="recip", bufs=2,
                                    name=f"recip_{h}_{jc}")
                    nc.vector.reciprocal(recip, ps_den)
                    if FP8C:
                        of32 = pb.tile([128, MC], F32, tag="of32", bufs=2,
                                       name=f"of32_{h}_{jc}")
                        nc.vector.tensor_mul(of32, ps_o, recip)
                        hs = othi[:, h, qs]
                        nc.scalar.activation(hs, of32, AF.Copy)
                        ores = pb.tile([128, MC], F32, tag="ores", bufs=2,
                                       name=f"ores_{h}_{jc}")
                        nc.vector.tensor_sub(ores, of32, hs)
                        nc.scalar.activation(otlo[:, h, qs], ores, AF.Copy,
                                             scale=16.0)
                    else:
                        nc.vector.tensor_mul(ot[h][:, qs], ps_o, recip)

            def C(j):
                for mt in range(4 * j, 4 * j + 4):
                    mts = slice(128 * mt, 128 * (mt + 1))
                    yrow = pb.tile([128, HID], F32, tag="yrow", bufs=YRB,
                                   name=f"yrow_{mt}")
                    last = mt == NMT - 1
                    for nb in range(8):
                        ps_y = pbp.tile([128, 512], F32, tag="y", bufs=YB,
                                        name=f"ps_y_{mt}_{nb}")
                        if FP8C:
                            for c in range(2):
                                l_hi = othi[:, 2 * c:2 * c + 2, mts]
                                l_lo = otlo[:, 2 * c:2 * c + 2, mts]
                                woff = ((c * 8 + nb) * 3) * 1024
                                w3 = [_pair3(wo_sb[:, woff + 1024 * v:
                                                   woff + 1024 * (v + 1)])
                                      for v in range(3)]
                                nc.tensor.matmul(ps_y, l_hi, w3[0],
                                                 start=(c == 0), stop=False,
                                                 perf_mode=DR)
                                nc.tensor.matmul(ps_y, l_hi, w3[1],
                                                 start=False, stop=False,
                                                 perf_mode=DR)
                                nc.tensor.matmul(ps_y, l_lo, w3[2],
                                                 start=False, stop=(c == 1),
                                                 perf_mode=DR)
                        else:
                            for dc in range(HPC):
                                wslice = wo_sb[:, 512 * (8 * dc + nb):
                                               512 * (8 * dc + nb + 1)]
                                nc.tensor.matmul(ps_y, ot[dc][:, mts], wslice,
                                                 start=(dc == 0),
                                                 stop=(dc == HPC - 1))
                        ys = slice(512 * nb, 512 * (nb + 1))
                        # split the PSUM drain across ACT and DVE
                        if nb % 2 == 0:
                            nc.scalar.activation(yrow[:, ys], ps_y, AF.Copy,
                                                 scale=(1.0 / WS) if FP8C
                                                 else 1.0)
                        elif FP8C:
                            nc.vector.tensor_scalar_mul(yrow[:, ys], ps_y,
                                                        1.0 / WS)
                        else:
                            nc.vector.tensor_copy(yrow[:, ys], ps_y)
                        if last and nb % 2 == 1:
                            # drip the final row out as it completes so the
                            # kernel tail isn't one long DMA
                            qq = nb // 2
                            nc.sync.dma_start(y[mts, 1024 * qq:1024 * (qq + 1)],
                                              yrow[:, 1024 * qq:1024 * (qq + 1)])
                    if not last:
                        nc.sync.dma_start(y[mts, :], yrow)

            B(0)
            B(1)
            if "C" in phases:
                C(0)
            B(2)
            if "C" in phases:
                C(1)
            B(3)
            if "C" in phases:
                C(2)
                C(3)


_BUILT = None


def _build():
    global _BUILT
    if _BUILT is not None:
        return _BUILT
    nc = bacc.Bacc("TRN2", target_bir_lowering=False, debug=False,
                   num_devices=NCORES)
    if FP8A:
        xkv = nc.dram_tensor("xkv", [128, NJC * NPAIR * PBLK], FP8,
                             kind="ExternalInput").ap()
        wq = nc.dram_tensor("wq", [128, NPAIR * 3072], FP8,
                            kind="ExternalInput").ap()
    else:
        xkv = nc.dram_tensor("xkv", [128, NJC * NKC * FB], BF16,
                             kind="ExternalInput").ap()
        wq = nc.dram_tensor("wq", [128, NKC * MC], BF16,
                            kind="ExternalInput").ap()
    if FP8C:
        wo = nc.dram_tensor("wo", [128, 2 * 8 * 3 * 1024], FP8,
                            kind="ExternalInput").ap()
    else:
        wo = nc.dram_tensor("wo", [128, HPC * 8 * 512], BF16,
                            kind="ExternalInput").ap()
    cosT = nc.dram_tensor("cosT", [D, S], BF16, kind="ExternalInput").ap()
    sinT = nc.dram_tensor("sinT", [D, S], BF16, kind="ExternalInput").ap()
    maskT = nc.dram_tensor("maskT", [MC, MC], BF16, kind="ExternalInput").ap()
    y = nc.dram_tensor("y", [S, HID], F32, kind="ExternalOutput").ap()
    with tile.TileContext(nc) as tc:
        _emit(nc, xkv, wq, wo, cosT, sinT, maskT, y, tc)
    nc.compile()
    _BUILT = nc
    return nc


def _fp8_split(m64):
    """m64: [rows, cols] f32 (already weight-scaled). Returns hi, lo, hi/16
    as fp8 arrays."""
    hi = m64.astype(NP_FP8)
    hif = hi.astype(np.float32)
    lo = (m64 - hif).astype(NP_FP8)
    hi16 = (hif / 16.0).astype(NP_FP8)
    return hi, lo, hi16


def _pairify(m):
    """[4096, W] -> [NPAIR, 128, 2, W]: [p, part, j, n] = m[256p+128j+part, n]."""
    return m.reshape(NPAIR, 2, 128, -1).transpose(0, 2, 1, 3)


def prep_in_maps(hidden_states, Wq, Wk, Wv, Wo, attention_mask, position_ids):
    hidden_states = np.asarray(hidden_states, dtype=np.float32)
    Wq = np.asarray(Wq, dtype=np.float32)
    Wk = np.asarray(Wk, dtype=np.float32)
    Wv = np.asarray(Wv, dtype=np.float32)
    Wo = np.asarray(Wo, dtype=np.float32)
    attention_mask = np.asarray(attention_mask, dtype=np.float32)
    position_ids = np.asarray(position_ids)

    xT = np.ascontiguousarray(hidden_states[0].T)  # [HID, S] f32

    # RoPE tables (host-precomputed from position_ids, as in the reference)
    pos = position_ids[0].astype(np.float32)  # [S]
    inv_freq = (1.0 / (ROPE_THETA ** (np.arange(0, D, 2, dtype=np.float32) / D))
                ).astype(np.float32)
    freqs = pos[:, None] * inv_freq[None, :]           # [S, D/2]
    emb = np.concatenate([freqs, freqs], axis=-1)      # [S, D]
    cosT = np.ascontiguousarray(np.cos(emb).T).astype(NP_BF16)  # [D, S]
    sinT = np.sin(emb).T.astype(np.float32)
    sinT = np.ascontiguousarray(
        np.concatenate([sinT[64:], sinT[:64]], axis=0)).astype(NP_BF16)

    # diagonal staircase keep-mask (1 = keep, 0 = masked), sliced from the
    # provided additive mask
    maskT = np.ascontiguousarray(
        (attention_mask[0, 0, :MC, :MC].T == 0.0)).astype(NP_BF16)  # [k, q]

    if FP8A:
        xhi = xT.astype(NP_FP8)
        xlo16 = ((xT - xhi.astype(np.float32)) * 16.0).astype(NP_FP8)
        xhi_p = _pairify(xhi)      # [p, part, j, S]
        xlo_p = _pairify(xlo16)
    else:
        xTr = xT.astype(NP_BF16).reshape(NKC, 128, S)

    in_maps = []
    for c in range(NCORES):
        if FP8A:
            # xkv pair blocks: [mc, pair, part, PBLK]
            blocks = np.zeros((NJC, NPAIR, 128, PBLK), dtype=NP_FP8)
            for mc in range(NJC):
                msl = slice(MC * mc, MC * (mc + 1))
                blocks[mc, :, :, XHI:XHI + 1024] = (
                    xhi_p[:, :, :, msl].reshape(NPAIR, 128, 1024))
                blocks[mc, :, :, XLO:XLO + 1024] = (
                    xlo_p[:, :, :, msl].reshape(NPAIR, 128, 1024))
            for base, W in ((KW, Wk), (VW, Wv)):
                w3 = _fp8_split(WS * W[:, D * c:D * (c + 1)])
                for v in range(3):
                    wp = _pairify(w3[v].astype(np.float32)).astype(NP_FP8)
                    blocks[:, :, :, base + 256 * v:base + 256 * (v + 1)] = (
                        wp.reshape(NPAIR, 128, 256)[None])
            xkv_c = blocks.transpose(2, 0, 1, 3).reshape(128, -1)
            # wq: [pair, part, v, j, 512]
            wq3 = _fp8_split(WS * Wq[:, DPC * c:DPC * (c + 1)])
            wq_c = np.stack(
                [_pairify(w.astype(np.float32)).astype(NP_FP8) for w in wq3],
                axis=2)  # [p, part, v, j, 512]
            wq_c = wq_c.transpose(1, 0, 2, 3, 4).reshape(128, -1)
        else:
            wk_c = Wk[:, D * c:D * (c + 1)].astype(NP_BF16).reshape(NKC, 128, D)
            wv_c = Wv[:, D * c:D * (c + 1)].astype(NP_BF16).reshape(NKC, 128, D)
            blocks = np.empty((NJC, NKC, 128, FB), dtype=NP_BF16)
            for mc in range(NJC):
                blocks[mc, :, :, :MC] = xTr[:, :, MC * mc:MC * (mc + 1)]
                blocks[mc, :, :, MC:MC + D] = wk_c
                blocks[mc, :, :, MC + D:] = wv_c
            xkv_c = blocks.transpose(2, 0, 1, 3).reshape(128, -1)
            wq_c = (Wq[:, DPC * c:DPC * (c + 1)].astype(NP_BF16)
                    .reshape(NKC, 128, DPC).transpose(1, 0, 2).reshape(128, -1))

        Wo_c = Wo[DPC * c:DPC * (c + 1), :]  # [512, 4096]
        if FP8C:
            wo3 = _fp8_split(WS * Wo_c)  # each [512, 4096]
            # [part, c, nb, v, j, 512]
            wo_c = np.zeros((128, 2, 8, 3, 2, 512), dtype=NP_FP8)
            for v in range(3):
                wp = (wo3[v].astype(np.float32)
                      .reshape(2, 2, 128, 8, 512))  # [c, j, part, nb, n]
                wo_c[:, :, :, v] = wp.transpose(2, 0, 3, 1, 4).astype(NP_FP8)
            wo_c = wo_c.reshape(128, -1)
        else:
            wo_c = (Wo_c.astype(NP_BF16)
                    .reshape(HPC, 128, 8, 512).transpose(1, 0, 2, 3)
                    .reshape(128, -1))
        in_maps.append({
            "xkv": np.ascontiguousarray(xkv_c),
            "wq": np.ascontiguousarray(wq_c),
            "wo": np.ascontiguousarray(wo_c),
            "cosT": cosT,
            "sinT": sinT,
            "maskT": maskT,
        })

    return in_maps


def kernel(hidden_states, Wq, Wk, Wv, Wo, attention_mask, position_ids):
    global LAST_RESULTS
    in_maps = prep_in_maps(hidden_states, Wq, Wk, Wv, Wo, attention_mask,
                           position_ids)
    nc = _build()
    res = run_bass_kernel_spmd(nc, in_maps, list(range(NCORES)),
                               trace=bool(int(os.environ.get("KERNEL_TRACE", "0"))))
    LAST_RESULTS = res

    acc = np.zeros((S, HID), dtype=np.float64)
    for c in range(NCORES):
        acc += res.results[c]["y"].astype(np.float64)
    return acc.astype(np.float32)[None]  # [1, S, HID]
